# revision 7
# baseline (speedup 1.0000x reference)
"""Trainium2 Bass kernel for nn_MemoryLayerAttention_27917287424099.

Mathematical collapse of the reference RNN:
  - The conductance-ODE "pot" state receives zero external input
    (neuron_inputs = zeros), starts at the same (0, 1) pair in every one
    of the BQ*MC cells, and its update depends only on itself and
    hardcoded constants.  It therefore evolves identically in every cell
    and is a compile-time-constant scalar trajectory.
  - Only the LAST scan step's LSTM output is returned (ys[-1]), and steps
    interact only through pot, so steps 0..6's attention/LSTM outputs are
    dead code.
  - Hence the whole model == one attention + LSTM-gate step evaluated on
    x_7 = concat(queries[b,q], values[b,7]) with the memory matrix equal
    to the constant p0 (pot[...,0] after 7*2 Euler iterations) broadcast
    everywhere.
  - Of the LSTM gate pre-activation z (4*1184 cols), only zi/zg/zo's
    first 1024 columns are used (zf and the tail are dead).

Sharding: batch (128) lives on the SBUF partition dim; the replicated
attention preamble is computed on every core, and the 1024 output
columns of the LSTM matmul + gate math are sharded 128/core across the
8 cores (each core gets its own 3*128-column slice of Wx/bl).

HW gotcha encoded here: each independent matmul accumulation group MUST
own its own PSUM tile (bank).  Two groups writing disjoint slices of one
PSUM bank crash the device.
"""

import os
import numpy as np

DIM = 16
EMB = 64
ROWS = 64
RH = 2
OUT = 1024
UNITS = 1184
B, Q, V = 8, 16, 8
BQ = B * Q
DSTEPS = 2
N_CORES = 8
CPC = OUT // N_CORES  # columns per core = 128
SCALE = float(1.0 / np.sqrt(np.float32(EMB)))

# ---------------------------------------------------------------------------
# compile-time constants (derived only from constants hardcoded in the model)
# ---------------------------------------------------------------------------


def _pot_scalar():
    """p0 = pot[..., 0] as read by scan step 7 (after 14 f32 Euler steps)."""
    cond = np.array([0.07915332, 1.0334609, 1.3365093, 0.4505964], np.float32)
    mean = np.array([0.5, 0.07879465, 0.06618887, 0.0], np.float32)
    std = np.array([100.0, 100.0, 100.0, 1.0], np.float32)
    tgt = np.array([1.5931877, 1.4378392, 0.0, 0.0], np.float32)
    part = np.float32(1.5573331 / DSTEPS)

    def sig(x):
        return np.float32(1.0) / (np.float32(1.0) + np.exp(-x, dtype=np.float32))

    p = np.array([0.0, 1.0], np.float32)
    inp = np.zeros(2, np.float32)
    for _ in range((V - 1) * DSTEPS):
        pre = np.stack([inp, p, p[::-1], np.full_like(p, np.inf)], -1)
        s = sig(std * (pre - mean))
        curr = cond * s * (tgt - p[:, None])
        p = (p + curr.sum(-1, dtype=np.float32) * part).astype(np.float32)
    return float(p[0])


P0 = _pot_scalar()


def _pe_table():
    L = ROWS + 1
    pos = np.arange(L, dtype=np.float32)[:, None]
    i = np.arange(EMB)[None, :]
    ang = pos / np.power(10000.0, (2 * (i // 2)) / EMB)
    return np.where(i % 2 == 0, np.sin(ang), np.cos(ang)).astype(np.float32)


PE = _pe_table()  # (65, 64)

# ---------------------------------------------------------------------------
# bass graph (built once; shapes are static)
# ---------------------------------------------------------------------------

_CACHE = {}
LAST_EXEC_TIME_NS = None


def _build():
    import concourse.bacc as bacc
    import concourse.tile as tile
    from concourse import mybir

    F32 = mybir.dt.float32
    AF = mybir.ActivationFunctionType
    AX = mybir.AxisListType

    nc = bacc.Bacc(None, target_bir_lowering=False, debug=False)

    # inputs (identical on all cores except WxA)
    d_x7aT = nc.declare_dram_parameter("x7aT", [33, BQ], F32, isOutput=False)
    d_WiA = nc.declare_dram_parameter("WiA", [33, EMB], F32, isOutput=False)
    d_WqA = nc.declare_dram_parameter("WqA", [EMB + 1, 128], F32, isOutput=False)
    d_WkA = nc.declare_dram_parameter("WkA", [EMB + 1, 128], F32, isOutput=False)
    d_WvA = nc.declare_dram_parameter("WvA", [EMB + 1, 128], F32, isOutput=False)
    d_PET1 = nc.declare_dram_parameter("PET1", [EMB, ROWS], F32, isOutput=False)
    d_WmP = nc.declare_dram_parameter("WmP", [128, 2, EMB], F32, isOutput=False)
    d_bm = nc.declare_dram_parameter("bm", [EMB, 1], F32, isOutput=False)
    d_WoP = nc.declare_dram_parameter("WoP", [EMB, 2, EMB], F32, isOutput=False)
    d_bo = nc.declare_dram_parameter("bo", [EMB, 1], F32, isOutput=False)
    d_ident = nc.declare_dram_parameter("ident", [128, 128], F32, isOutput=False)
    d_ones = nc.declare_dram_parameter("ones", [128, 1], F32, isOutput=False)
    d_hmask = nc.declare_dram_parameter("hmask", [128, RH], F32, isOutput=False)
    d_WxA = nc.declare_dram_parameter("WxA", [EMB + 1, 3 * CPC], F32, isOutput=False)
    d_out = nc.declare_dram_parameter("out", [BQ, CPC], F32, isOutput=True)

    with tile.TileContext(nc) as tc:
        with (
            tc.tile_pool(name="sb", bufs=1) as sb,
            tc.tile_pool(name="ps", bufs=1, space="PSUM") as ps,
        ):
            # ---- loads -------------------------------------------------
            def load(name, dram, shape):
                t = sb.tile(shape, F32, tag=name, name=name)
                nc.sync.dma_start(out=t[:], in_=dram[:])
                return t

            x7aT = load("x7aT", d_x7aT, [33, BQ])
            WiA = load("WiA", d_WiA, [33, EMB])
            WqA = load("WqA", d_WqA, [EMB + 1, 128])
            WkA = load("WkA", d_WkA, [EMB + 1, 128])
            WvA = load("WvA", d_WvA, [EMB + 1, 128])
            PET1 = load("PET1", d_PET1, [EMB, ROWS])
            WmP = load("WmP", d_WmP, [128, 2, EMB])
            bm = load("bm", d_bm, [EMB, 1])
            WoP = load("WoP", d_WoP, [EMB, 2, EMB])
            bo = load("bo", d_bo, [EMB, 1])
            ident = load("ident", d_ident, [128, 128])
            ones = load("ones", d_ones, [128, 1])
            hmask = load("hmask", d_hmask, [128, RH])
            WxA = load("WxA", d_WxA, [EMB + 1, 3 * CPC])

            # warm the ACT table set early (Exp table load overlaps the DMAs)
            warm = sb.tile([128, 1], F32, tag="warm", name="warm")
            nc.vector.memset(warm[:], 0.0)
            warm2 = sb.tile([128, 1], F32, tag="warm2", name="warm2")
            nc.scalar.activation(warm2[:], warm[:], AF.Exp)

            # ---- aug0T = (x7 @ Wi + bi + PE0)^T, augmented with ones row
            emb_ps = ps.tile([EMB, BQ], F32, tag="mm", bufs=5, name="emb_ps")
            nc.tensor.matmul(emb_ps[:], lhsT=WiA[:], rhs=x7aT[:], start=True, stop=True)
            aug0T = sb.tile([EMB + 1, BQ], F32, tag="aug0T", name="aug0T")
            nc.scalar.copy(aug0T[0:EMB, :], emb_ps[:])
            nc.vector.memset(aug0T[EMB : EMB + 1, :], 1.0)

            # ---- m_vec = p0 * colsum(Wm) + bm  (per-partition, EMB rows)
            colsum_ps = ps.tile([EMB, 1], F32, tag="mm", bufs=5, name="colsum_ps")
            nc.tensor.matmul(
                colsum_ps[:], lhsT=WmP[:, 0, :], rhs=ones[:, :], start=True, stop=False
            )
            nc.tensor.matmul(
                colsum_ps[:], lhsT=WmP[:, 1, :], rhs=ones[:, :], start=False, stop=True
            )
            m_vec = sb.tile([EMB, 1], F32, tag="m_vec", name="m_vec")
            nc.scalar.activation(
                m_vec[:], colsum_ps[:], AF.Identity, bias=bm[:], scale=P0
            )

            # ---- augR = (m_vec + PE[1:].T), augmented with ones row -----
            augR = sb.tile([EMB + 1, ROWS], F32, tag="augR", name="augR")
            nc.vector.tensor_scalar_add(augR[0:EMB, :], PET1[:], m_vec[:])
            nc.vector.memset(augR[EMB : EMB + 1, :], 1.0)

            # ---- q / k0 / v0 -------------------------------------------
            q_ps = ps.tile([128, BQ], F32, tag="mm", bufs=5, name="q_ps")
            nc.tensor.matmul(q_ps[:], lhsT=WqA[:], rhs=aug0T[:], start=True, stop=True)
            qT = sb.tile([128, BQ], F32, tag="qT", name="qT")
            nc.scalar.mul(qT[:], q_ps[:], SCALE)  # fold attention scale into q

            k0_ps = ps.tile([128, BQ], F32, tag="mm", bufs=5, name="k0_ps")
            nc.tensor.matmul(k0_ps[:], lhsT=WkA[:], rhs=aug0T[:], start=True, stop=True)
            k0T = sb.tile([128, BQ], F32, tag="k0T", name="k0T")
            nc.scalar.copy(k0T[:], k0_ps[:])

            # v0 batch-major: (128b, 128hk)
            v0_ps = ps.tile([BQ, 128], F32, tag="mm", bufs=5, name="v0_ps")
            nc.tensor.matmul(v0_ps[:], lhsT=aug0T[:], rhs=WvA[:], start=True, stop=True)
            v0bm = sb.tile([BQ, 128], F32, tag="v0bm", name="v0bm")
            nc.vector.tensor_copy(v0bm[:], v0_ps[:])

            # ---- K^T (k-major) and V (l-major) for the 64 memory rows ---
            kt_ps = ps.tile([128, ROWS], F32, tag="mm", bufs=5, name="kt_ps")
            nc.tensor.matmul(kt_ps[:], lhsT=WkA[:], rhs=augR[:], start=True, stop=True)
            ktT = sb.tile([128, ROWS], F32, tag="ktT", name="ktT")
            nc.scalar.copy(ktT[:], kt_ps[:])

            vl_ps = ps.tile([ROWS, 128], F32, tag="mm", bufs=5, name="vl_ps")
            nc.tensor.matmul(vl_ps[:], lhsT=augR[:], rhs=WvA[:], start=True, stop=True)
            vl = sb.tile([ROWS, 128], F32, tag="vl", name="vl")
            nc.vector.tensor_copy(vl[:], vl_ps[:])

            # ---- attention logits --------------------------------------
            # rest-rows logits per head; one PSUM tile per matmul group!
            logR_ps = []
            for h in range(RH):
                lp = ps.tile([BQ, ROWS], F32, tag="mm", bufs=5, name=f"logR{h}")
                nc.tensor.matmul(
                    lp[:],
                    lhsT=qT[h * EMB : (h + 1) * EMB, :],
                    rhs=ktT[h * EMB : (h + 1) * EMB, :],
                    start=True,
                    stop=True,
                )
                logR_ps.append(lp)
            # row-0 logit, both heads in one masked matmul -> (128b, 2h)
            prod = sb.tile([128, BQ], F32, tag="prod", name="prod")
            nc.vector.tensor_mul(prod[:], qT[:], k0T[:])
            log0_ps = ps.tile([BQ, RH], F32, tag="mm", bufs=5, name="log0_ps")
            nc.tensor.matmul(
                log0_ps[:], lhsT=prod[:], rhs=hmask[:], start=True, stop=True
            )

            # ---- softmax over 65 positions per (b, h) -------------------
            # |logit| <= ~2 here, so no max-subtraction needed before exp
            e = sb.tile([BQ, RH, ROWS + 1], F32, tag="e", name="e")
            for h in range(RH):
                nc.scalar.activation(e[:, h, 0:1], log0_ps[:, h : h + 1], AF.Exp)
                nc.scalar.activation(e[:, h, 1:], logR_ps[h][:], AF.Exp)
            ssum = sb.tile([BQ, RH], F32, tag="ssum", name="ssum")
            nc.vector.reduce_sum(ssum[:], e[:, :, :], axis=AX.X)
            rsum = sb.tile([BQ, RH], F32, tag="rsum", name="rsum")
            nc.vector.reciprocal(rsum[:], ssum[:])
            attn = sb.tile([BQ, RH, ROWS + 1], F32, tag="attn", name="attn")
            for h in range(RH):
                nc.vector.tensor_scalar_mul(
                    attn[:, h, :], e[:, h, :], rsum[:, h : h + 1]
                )

            # ---- ctx^T, laid out (64 k, 2 h, 128 b); all matmul operands
            # at base partition 0, one PSUM tile per matmul group ---------
            atT_sb = []
            for h in range(RH):
                atT_ps = ps.tile([ROWS, BQ], F32, tag="mm", bufs=5, name=f"atT{h}")
                nc.tensor.transpose(atT_ps[:], attn[:, h, 1:], ident[:])
                t = sb.tile([ROWS, BQ], F32, tag=f"atTs{h}", name=f"atTs{h}")
                nc.scalar.copy(t[:], atT_ps[:])
                atT_sb.append(t)
            ctxR_ps = []
            for h in range(RH):
                cp = ps.tile([EMB, BQ], F32, tag="ctx", bufs=2, name=f"ctxR{h}")
                nc.tensor.matmul(
                    cp[:],
                    lhsT=vl[:, h * EMB : (h + 1) * EMB],
                    rhs=atT_sb[h][:],
                    start=True,
                    stop=True,
                )
                ctxR_ps.append(cp)
            # l=0 term: attn0 * v0 batch-major, transposed per head
            ctx0bm = sb.tile([BQ, 128], F32, tag="ctx0bm", name="ctx0bm")
            for h in range(RH):
                nc.vector.tensor_scalar_mul(
                    ctx0bm[:, h * EMB : (h + 1) * EMB],
                    v0bm[:, h * EMB : (h + 1) * EMB],
                    attn[:, h, 0:1],
                )
            ctx0T_sb = sb.tile([EMB, RH, BQ], F32, tag="ctx0T_sb", name="ctx0T_sb")
            for h in range(RH):
                c0p = ps.tile([EMB, BQ], F32, tag="mm", bufs=5, name=f"ctx0T{h}")
                nc.tensor.transpose(c0p[:], ctx0bm[:, h * EMB : (h + 1) * EMB], ident[:])
                nc.scalar.copy(ctx0T_sb[:, h, :], c0p[:])
            ctx = sb.tile([EMB, RH, BQ], F32, tag="ctx_sb", name="ctx")
            for h in range(RH):
                nc.vector.tensor_add(ctx[:, h, :], ctxR_ps[h][:], ctx0T_sb[:, h, :])

            # ---- o^T = sum_h Wo[h]^T ctx[h] + bo, augmented ones row ----
            oT_ps = ps.tile([EMB, BQ], F32, tag="mm", bufs=5, name="oT_ps")
            for h in range(RH):
                nc.tensor.matmul(
                    oT_ps[:],
                    lhsT=WoP[:, h, :],
                    rhs=ctx[:, h, :],
                    start=(h == 0),
                    stop=(h == RH - 1),
                )
            oTa = sb.tile([EMB + 1, BQ], F32, tag="oTa", name="oTa")
            nc.scalar.activation(oTa[0:EMB, :], oT_ps[:], AF.Identity, bias=bo[:])
            nc.vector.memset(oTa[EMB : EMB + 1, :], 1.0)

            # ---- z = o @ WxA + bl  (this core's 3*128 columns) ----------
            z_ps = ps.tile([BQ, 3 * CPC], F32, tag="z", bufs=1, name="z_ps")
            nc.tensor.matmul(z_ps[:], lhsT=oTa[:], rhs=WxA[:], start=True, stop=True)

            # ---- gates: out = sig(zo) * tanh(sig(zi) * tanh(zg)) --------
            sig_i = sb.tile([BQ, CPC], F32, tag="sig_i", name="sig_i")
            nc.scalar.activation(sig_i[:], z_ps[:, 0:CPC], AF.Sigmoid)
            tanh_g = sb.tile([BQ, CPC], F32, tag="tanh_g", name="tanh_g")
            nc.scalar.activation(tanh_g[:], z_ps[:, CPC : 2 * CPC], AF.Tanh)
            sig_o = sb.tile([BQ, CPC], F32, tag="sig_o", name="sig_o")
            nc.scalar.activation(sig_o[:], z_ps[:, 2 * CPC : 3 * CPC], AF.Sigmoid)
            c_sb = sb.tile([BQ, CPC], F32, tag="c_sb", name="c_sb")
            nc.vector.tensor_mul(c_sb[:], sig_i[:], tanh_g[:])
            tanh_c = sb.tile([BQ, CPC], F32, tag="tanh_c", name="tanh_c")
            nc.scalar.activation(tanh_c[:], c_sb[:], AF.Tanh)
            out_sb = sb.tile([BQ, CPC], F32, tag="out_sb", name="out_sb")
            nc.vector.tensor_mul(out_sb[:], sig_o[:], tanh_c[:])

            nc.sync.dma_start(out=d_out[:], in_=out_sb[:])

    nc.compile()
    return nc


def _get_nc():
    if "nc" not in _CACHE:
        _CACHE["nc"] = _build()
    return _CACHE["nc"]


# ---------------------------------------------------------------------------
# host-side packing + execution
# ---------------------------------------------------------------------------


def _pack_common(queries, values, Wi, bi, Wm, bm, Wq, bq, Wk, bk, Wv, bv, Wo, bo):
    f = np.float32
    queries = np.asarray(queries, f)
    values = np.asarray(values, f)

    # x_7 = concat(queries[b,q], values[b,7]) for row b*Q+q, transposed+ones row
    x7 = np.concatenate(
        [queries.reshape(BQ, DIM), np.repeat(values[:, V - 1, :], Q, axis=0)], axis=1
    )
    x7aT = np.concatenate([x7.T, np.ones((1, BQ), f)], axis=0)

    WiA = np.concatenate([np.asarray(Wi, f), (np.asarray(bi, f) + PE[0])[None, :]], 0)
    WqA = np.concatenate(
        [np.asarray(Wq, f).reshape(EMB, 128), np.asarray(bq, f).reshape(1, 128)], 0
    )
    WkA = np.concatenate(
        [np.asarray(Wk, f).reshape(EMB, 128), np.asarray(bk, f).reshape(1, 128)], 0
    )
    WvA = np.concatenate(
        [np.asarray(Wv, f).reshape(EMB, 128), np.asarray(bv, f).reshape(1, 128)], 0
    )
    PET1 = np.ascontiguousarray(PE[1:].T)  # (64 d, 64 l)
    WmP = np.ascontiguousarray(
        np.asarray(Wm, f).reshape(2, 128, EMB).transpose(1, 0, 2)
    )  # (128, 2, 64): [:, c, :] = Wm[c*128:(c+1)*128]
    WoP = np.ascontiguousarray(np.asarray(Wo, f).transpose(1, 0, 2))  # (64k, 2h, 64d)
    hmask = np.zeros((128, RH), f)
    for h in range(RH):
        hmask[h * EMB : (h + 1) * EMB, h] = 1.0

    return {
        "x7aT": np.ascontiguousarray(x7aT.astype(f)),
        "WiA": np.ascontiguousarray(WiA.astype(f)),
        "WqA": np.ascontiguousarray(WqA.astype(f)),
        "WkA": np.ascontiguousarray(WkA.astype(f)),
        "WvA": np.ascontiguousarray(WvA.astype(f)),
        "PET1": PET1.astype(f),
        "WmP": WmP.astype(f),
        "bm": np.ascontiguousarray(np.asarray(bm, f).reshape(EMB, 1)),
        "WoP": WoP.astype(f),
        "bo": np.ascontiguousarray(np.asarray(bo, f).reshape(EMB, 1)),
        "ident": np.eye(128, dtype=f),
        "ones": np.ones((128, 1), f),
        "hmask": hmask,
    }


def kernel(
    queries,
    values,
    Wi,
    bi,
    Wm,
    bm,
    Wq,
    bq,
    Wk,
    bk,
    Wv,
    bv,
    Wo,
    bo,
    Wx,
    bl,
):
    global LAST_EXEC_TIME_NS
    from concourse.bass_utils import run_bass_kernel_spmd

    f = np.float32
    common = _pack_common(
        queries, values, Wi, bi, Wm, bm, Wq, bq, Wk, bk, Wv, bv, Wo, bo
    )
    Wx = np.asarray(Wx, f)
    bl = np.asarray(bl, f)

    # per-core slice of Wx/bl: zi, zg, zo gate blocks, CPC columns each
    gate_off = [0, 2 * UNITS, 3 * UNITS]  # zi, zg, zo starts in the 4*UNITS axis
    in_maps = []
    for c in range(N_CORES):
        cols = np.concatenate(
            [np.arange(off + c * CPC, off + (c + 1) * CPC) for off in gate_off]
        )
        WxA = np.concatenate([Wx[:, cols], bl[cols][None, :]], axis=0).astype(f)
        in_maps.append({**common, "WxA": np.ascontiguousarray(WxA)})

    nc = _get_nc()
    trace = os.environ.get("BASS_TRACE", "") not in ("", "0")
    core_ids = list(range(N_CORES))
    if trace:
        import tempfile

        tmpdir = tempfile.mkdtemp(prefix="bass_trace_")
        _CACHE["trace_dir"] = tmpdir
        try:
            res = run_bass_kernel_spmd(
                nc, in_maps, core_ids=core_ids, trace=True, tmpdir=tmpdir
            )
        except Exception as e:  # profiling infra missing: fall back untraced
            print(f"trace failed ({e!r}); rerunning without trace")
            os.environ["BASS_TRACE"] = "0"
            res = run_bass_kernel_spmd(nc, in_maps, core_ids=core_ids, trace=False)
    else:
        res = run_bass_kernel_spmd(nc, in_maps, core_ids=core_ids, trace=False)
    LAST_EXEC_TIME_NS = res.exec_time_ns

    out_full = np.concatenate([res.results[c]["out"] for c in range(N_CORES)], axis=1)
    return out_full.reshape(-1, Q, DIM).astype(f)


# revision 10
# speedup vs baseline: 1.3840x; 1.3840x over previous
"""Trainium2 Bass kernel for nn_MemoryLayerAttention_27917287424099.

Mathematical collapse of the reference RNN:
  - The conductance-ODE "pot" state receives zero external input
    (neuron_inputs = zeros), starts at the same (0, 1) pair in every one
    of the BQ*MC cells, and its update depends only on itself and
    hardcoded constants.  It therefore evolves identically in every cell
    and is a compile-time-constant scalar trajectory.
  - Only the LAST scan step's LSTM output is returned (ys[-1]), and steps
    interact only through pot, so steps 0..6's attention/LSTM outputs are
    dead code.
  - Hence the whole model == one attention + LSTM-gate step evaluated on
    x_7 = concat(queries[b,q], values[b,7]) with the memory matrix equal
    to the constant p0 (pot[...,0] after 7*2 Euler iterations) broadcast
    everywhere.
  - Of the LSTM gate pre-activation z (4*1184 cols), only zi/zg/zo's
    first 1024 columns are used (zf and the tail are dead).

Sharding: batch (128) lives on the SBUF partition dim; the replicated
attention preamble is computed on every core, and the 1024 output
columns of the LSTM matmul + gate math are sharded 128/core across the
8 cores (each core gets its own 3*128-column slice of Wx/bl).

Perf notes baked in:
  - fp32 matmuls run as LOW_HIGH double passes on trn2; all TensorE
    operands are bf16 here (single pass), PSUM accumulation stays fp32.
    Measured end-to-end error vs the f32 reference: ~5e-3.
  - each independent matmul accumulation group owns its own PSUM tile
    (two groups sharing a PSUM bank crash the device).
  - inputs arrive in 5 packed DMAs (DMA issue is serialized on SyncE at
    ~0.7us apiece, so count matters, not bytes).
  - sigmoid(x) = 0.5*(1+tanh(x/2)) keeps every ACT function in the
    exp_and_others table set: one ACT_TABLE_LOAD instead of two.
"""

import os
import numpy as np
import ml_dtypes

BF16 = ml_dtypes.bfloat16

DIM = 16
EMB = 64
ROWS = 64
RH = 2
OUT = 1024
UNITS = 1184
B, Q, V = 8, 16, 8
BQ = B * Q
DSTEPS = 2
N_CORES = 8
CPC = OUT // N_CORES  # columns per core = 128
SCALE = float(1.0 / np.sqrt(np.float32(EMB)))

# ---------------------------------------------------------------------------
# compile-time constants (derived only from constants hardcoded in the model)
# ---------------------------------------------------------------------------


def _pot_scalar():
    """p0 = pot[..., 0] as read by scan step 7 (after 14 f32 Euler steps)."""
    cond = np.array([0.07915332, 1.0334609, 1.3365093, 0.4505964], np.float32)
    mean = np.array([0.5, 0.07879465, 0.06618887, 0.0], np.float32)
    std = np.array([100.0, 100.0, 100.0, 1.0], np.float32)
    tgt = np.array([1.5931877, 1.4378392, 0.0, 0.0], np.float32)
    part = np.float32(1.5573331 / DSTEPS)

    def sig(x):
        return np.float32(1.0) / (np.float32(1.0) + np.exp(-x, dtype=np.float32))

    p = np.array([0.0, 1.0], np.float32)
    inp = np.zeros(2, np.float32)
    for _ in range((V - 1) * DSTEPS):
        pre = np.stack([inp, p, p[::-1], np.full_like(p, np.inf)], -1)
        s = sig(std * (pre - mean))
        curr = cond * s * (tgt - p[:, None])
        p = (p + curr.sum(-1, dtype=np.float32) * part).astype(np.float32)
    return float(p[0])


P0 = _pot_scalar()


def _pe_table():
    L = ROWS + 1
    pos = np.arange(L, dtype=np.float32)[:, None]
    i = np.arange(EMB)[None, :]
    ang = pos / np.power(10000.0, (2 * (i // 2)) / EMB)
    return np.where(i % 2 == 0, np.sin(ang), np.cos(ang)).astype(np.float32)


PE = _pe_table()  # (65, 64)

# packed-input column offsets
# pk33 (33, 192): x7aT | WiA
# pk65 (65, 768): WqA | WkA | WvA | WxA(384)
# pk64 (64, 192): PET1 | WoP_h0 | WoP_h1
# pk128 (128, 259): Wm_chunk0 | Wm_chunk1 | ident | ones | hmask(2)
# pkb  (64, 2) f32: bm | bo

_CACHE = {}
LAST_EXEC_TIME_NS = None


def _build():
    import concourse.bacc as bacc
    import concourse.tile as tile
    from concourse import mybir

    F32 = mybir.dt.float32
    BF = mybir.dt.bfloat16
    AF = mybir.ActivationFunctionType
    ALU = mybir.AluOpType
    AX = mybir.AxisListType

    nc = bacc.Bacc(None, target_bir_lowering=False, debug=False)

    d_pk33 = nc.declare_dram_parameter("pk33", [33, 192], BF, isOutput=False)
    d_pk65 = nc.declare_dram_parameter("pk65", [EMB + 1, 768], BF, isOutput=False)
    d_pk64 = nc.declare_dram_parameter("pk64", [EMB, 192], BF, isOutput=False)
    d_pk128 = nc.declare_dram_parameter("pk128", [128, 259], BF, isOutput=False)
    d_pkb = nc.declare_dram_parameter("pkb", [EMB, 2], F32, isOutput=False)
    d_out = nc.declare_dram_parameter("out", [BQ, CPC], F32, isOutput=True)

    with tile.TileContext(nc) as tc:
        with (
            tc.tile_pool(name="sb", bufs=1) as sb,
            tc.tile_pool(name="ps", bufs=1, space="PSUM") as ps,
        ):
            # ---- packed loads, ordered by first use --------------------
            pk33 = sb.tile([33, 192], BF, tag="pk33", name="pk33")
            nc.sync.dma_start(out=pk33[:], in_=d_pk33[:])
            pk65 = sb.tile([EMB + 1, 768], BF, tag="pk65", name="pk65")
            nc.sync.dma_start(out=pk65[:], in_=d_pk65[:])
            pk128 = sb.tile([128, 259], BF, tag="pk128", name="pk128")
            nc.sync.dma_start(out=pk128[:], in_=d_pk128[:])
            pkb = sb.tile([EMB, 2], F32, tag="pkb", name="pkb")
            nc.sync.dma_start(out=pkb[:], in_=d_pkb[:])
            pk64 = sb.tile([EMB, 192], BF, tag="pk64", name="pk64")
            nc.sync.dma_start(out=pk64[:], in_=d_pk64[:])

            x7aT = pk33[:, 0:128]
            WiA = pk33[:, 128:192]
            WqA = pk65[:, 0:128]
            WkA = pk65[:, 128:256]
            WvA = pk65[:, 256:384]
            WxA = pk65[:, 384:768]
            PET1 = pk64[:, 0:64]
            WoP = [pk64[:, 64 + h * EMB : 64 + (h + 1) * EMB] for h in range(RH)]
            WmC = [pk128[:, h * EMB : (h + 1) * EMB] for h in range(2)]
            ident = pk128[:, 128:256]
            ones = pk128[:, 256:257]
            hmask = pk128[:, 257:259]
            bm = pkb[:, 0:1]
            bo = pkb[:, 1:2]

            # warm the ACT table set early (Exp/Tanh load overlaps the DMAs)
            warm = sb.tile([128, 1], F32, tag="warm", name="warm")
            nc.vector.memset(warm[:], 0.0)
            warm2 = sb.tile([128, 1], F32, tag="warm2", name="warm2")
            nc.scalar.activation(warm2[:], warm[:], AF.Exp)

            # ---- aug0T = (x7 @ Wi + bi + PE0)^T, augmented with ones row
            emb_ps = ps.tile([EMB, BQ], F32, tag="mm", bufs=5, name="emb_ps")
            nc.tensor.matmul(emb_ps[:], lhsT=WiA, rhs=x7aT, start=True, stop=True)
            aug0T = sb.tile([EMB + 1, BQ], BF, tag="aug0T", name="aug0T")
            nc.scalar.copy(aug0T[0:EMB, :], emb_ps[:])
            nc.vector.memset(aug0T[EMB : EMB + 1, :], 1.0)

            # ---- m_vec = p0 * colsum(Wm) + bm  (per-partition, EMB rows)
            colsum_ps = ps.tile([EMB, 1], F32, tag="mm", bufs=5, name="colsum_ps")
            nc.tensor.matmul(
                colsum_ps[:], lhsT=WmC[0], rhs=ones, start=True, stop=False
            )
            nc.tensor.matmul(
                colsum_ps[:], lhsT=WmC[1], rhs=ones, start=False, stop=True
            )
            m_vec = sb.tile([EMB, 1], F32, tag="m_vec", name="m_vec")
            nc.scalar.activation(
                m_vec[:], colsum_ps[:], AF.Identity, bias=bm, scale=P0
            )

            # ---- augR = (m_vec + PE[1:].T), augmented with ones row -----
            augR = sb.tile([EMB + 1, ROWS], BF, tag="augR", name="augR")
            nc.vector.tensor_scalar_add(augR[0:EMB, :], PET1, m_vec[:])
            nc.vector.memset(augR[EMB : EMB + 1, :], 1.0)

            # ---- q / k0 / v0 -------------------------------------------
            q_ps = ps.tile([128, BQ], F32, tag="mm", bufs=5, name="q_ps")
            nc.tensor.matmul(q_ps[:], lhsT=WqA, rhs=aug0T[:], start=True, stop=True)
            qT = sb.tile([128, BQ], BF, tag="qT", name="qT")
            nc.scalar.mul(qT[:], q_ps[:], SCALE)  # fold attention scale into q

            k0_ps = ps.tile([128, BQ], F32, tag="mm", bufs=5, name="k0_ps")
            nc.tensor.matmul(k0_ps[:], lhsT=WkA, rhs=aug0T[:], start=True, stop=True)
            k0T = sb.tile([128, BQ], BF, tag="k0T", name="k0T")
            nc.scalar.copy(k0T[:], k0_ps[:])

            # v0 batch-major: (128b, 128hk)
            v0_ps = ps.tile([BQ, 128], F32, tag="mm", bufs=5, name="v0_ps")
            nc.tensor.matmul(v0_ps[:], lhsT=aug0T[:], rhs=WvA, start=True, stop=True)
            v0bm = sb.tile([BQ, 128], BF, tag="v0bm", name="v0bm")
            nc.vector.tensor_copy(v0bm[:], v0_ps[:])

            # ---- K^T (k-major) and V (l-major) for the 64 memory rows ---
            kt_ps = ps.tile([128, ROWS], F32, tag="mm", bufs=5, name="kt_ps")
            nc.tensor.matmul(kt_ps[:], lhsT=WkA, rhs=augR[:], start=True, stop=True)
            ktT = sb.tile([128, ROWS], BF, tag="ktT", name="ktT")
            nc.scalar.copy(ktT[:], kt_ps[:])

            vl_ps = ps.tile([ROWS, 128], F32, tag="mm", bufs=5, name="vl_ps")
            nc.tensor.matmul(vl_ps[:], lhsT=augR[:], rhs=WvA, start=True, stop=True)
            vl = sb.tile([ROWS, 128], BF, tag="vl", name="vl")
            nc.vector.tensor_copy(vl[:], vl_ps[:])

            # ---- attention logits --------------------------------------
            logR_ps = []
            for h in range(RH):
                lp = ps.tile([BQ, ROWS], F32, tag="mm", bufs=5, name=f"logR{h}")
                nc.tensor.matmul(
                    lp[:],
                    lhsT=qT[h * EMB : (h + 1) * EMB, :],
                    rhs=ktT[h * EMB : (h + 1) * EMB, :],
                    start=True,
                    stop=True,
                )
                logR_ps.append(lp)
            prod = sb.tile([128, BQ], BF, tag="prod", name="prod")
            nc.vector.tensor_mul(prod[:], qT[:], k0T[:])
            log0_ps = ps.tile([BQ, RH], F32, tag="mm", bufs=5, name="log0_ps")
            nc.tensor.matmul(log0_ps[:], lhsT=prod[:], rhs=hmask, start=True, stop=True)

            # ---- softmax over 65 positions per (b, h) -------------------
            # |logit| <= ~2 here, so no max-subtraction needed before exp
            e = sb.tile([BQ, RH, ROWS + 1], F32, tag="e", name="e")
            for h in range(RH):
                nc.scalar.activation(e[:, h, 0:1], log0_ps[:, h : h + 1], AF.Exp)
                nc.scalar.activation(e[:, h, 1:], logR_ps[h][:], AF.Exp)
            ssum = sb.tile([BQ, RH], F32, tag="ssum", name="ssum")
            nc.vector.reduce_sum(ssum[:], e[:, :, :], axis=AX.X)
            rsum = sb.tile([BQ, RH], F32, tag="rsum", name="rsum")
            nc.vector.reciprocal(rsum[:], ssum[:])
            attn = sb.tile([BQ, RH, ROWS + 1], BF, tag="attn", name="attn")
            for h in range(RH):
                nc.vector.tensor_scalar_mul(
                    attn[:, h, :], e[:, h, :], rsum[:, h : h + 1]
                )

            # ---- ctx^T, laid out (64 k, 2 h, 128 b); all matmul operands
            # at base partition 0, one PSUM tile per matmul group ---------
            atT_sb = []
            for h in range(RH):
                atT_ps = ps.tile([ROWS, BQ], BF, tag="mm", bufs=5, name=f"atT{h}")
                nc.tensor.transpose(atT_ps[:], attn[:, h, 1:], ident)
                t = sb.tile([ROWS, BQ], BF, tag=f"atTs{h}", name=f"atTs{h}")
                nc.scalar.copy(t[:], atT_ps[:])
                atT_sb.append(t)
            ctxR_ps = []
            for h in range(RH):
                cp = ps.tile([EMB, BQ], F32, tag="ctx", bufs=2, name=f"ctxR{h}")
                nc.tensor.matmul(
                    cp[:],
                    lhsT=vl[:, h * EMB : (h + 1) * EMB],
                    rhs=atT_sb[h][:],
                    start=True,
                    stop=True,
                )
                ctxR_ps.append(cp)
            # l=0 term: attn0 * v0 batch-major, one full transpose
            ctx0bm = sb.tile([BQ, 128], BF, tag="ctx0bm", name="ctx0bm")
            for h in range(RH):
                nc.vector.tensor_scalar(
                    ctx0bm[:, h * EMB : (h + 1) * EMB],
                    v0bm[:, h * EMB : (h + 1) * EMB],
                    e[:, h, 0:1],
                    rsum[:, h : h + 1],
                    op0=ALU.mult,
                    op1=ALU.mult,
                )
            c0p = ps.tile([128, BQ], BF, tag="mm", bufs=5, name="ctx0T_ps")
            nc.tensor.transpose(c0p[:], ctx0bm[:], ident)
            ctx0T_sb = sb.tile([128, BQ], F32, tag="ctx0T_sb", name="ctx0T_sb")
            nc.scalar.copy(ctx0T_sb[:], c0p[:])
            ctx = sb.tile([EMB, RH, BQ], BF, tag="ctx_sb", name="ctx")
            for h in range(RH):
                nc.vector.tensor_add(
                    ctx[:, h, :],
                    ctxR_ps[h][:],
                    ctx0T_sb[h * EMB : (h + 1) * EMB, :],
                )

            # ---- o^T = sum_h Wo[h]^T ctx[h] + bo, augmented ones row ----
            oT_ps = ps.tile([EMB, BQ], F32, tag="mm", bufs=5, name="oT_ps")
            for h in range(RH):
                nc.tensor.matmul(
                    oT_ps[:],
                    lhsT=WoP[h],
                    rhs=ctx[:, h, :],
                    start=(h == 0),
                    stop=(h == RH - 1),
                )
            oTa = sb.tile([EMB + 1, BQ], BF, tag="oTa", name="oTa")
            nc.scalar.activation(oTa[0:EMB, :], oT_ps[:], AF.Identity, bias=bo)
            nc.vector.memset(oTa[EMB : EMB + 1, :], 1.0)

            # ---- z = o @ WxA + bl  (this core's 3*128 columns) ----------
            z_ps = ps.tile([BQ, 3 * CPC], F32, tag="z", bufs=1, name="z_ps")
            nc.tensor.matmul(z_ps[:], lhsT=oTa[:], rhs=WxA, start=True, stop=True)

            # ---- gates via tanh only (one ACT table set):
            # sig(x) = 0.5*(1+tanh(x/2))
            # out = sig(zo)*tanh(sig(zi)*tanh(zg))
            #     = 0.5*(t_o+1)*tanh(0.5*(t_i+1)*t_g)
            t_i = sb.tile([BQ, CPC], F32, tag="t_i", name="t_i")
            nc.scalar.activation(t_i[:], z_ps[:, 0:CPC], AF.Tanh, scale=0.5)
            t_g = sb.tile([BQ, CPC], F32, tag="t_g", name="t_g")
            nc.scalar.activation(t_g[:], z_ps[:, CPC : 2 * CPC], AF.Tanh)
            t_o = sb.tile([BQ, CPC], F32, tag="t_o", name="t_o")
            nc.scalar.activation(t_o[:], z_ps[:, 2 * CPC : 3 * CPC], AF.Tanh, scale=0.5)
            c2 = sb.tile([BQ, CPC], F32, tag="c2", name="c2")
            nc.vector.scalar_tensor_tensor(
                c2[:], t_i[:], 1.0, t_g[:], op0=ALU.add, op1=ALU.mult
            )
            tanh_c = sb.tile([BQ, CPC], F32, tag="tanh_c", name="tanh_c")
            nc.scalar.activation(tanh_c[:], c2[:], AF.Tanh, scale=0.5)
            out2 = sb.tile([BQ, CPC], F32, tag="out2", name="out2")
            nc.vector.scalar_tensor_tensor(
                out2[:], t_o[:], 1.0, tanh_c[:], op0=ALU.add, op1=ALU.mult
            )
            out_sb = sb.tile([BQ, CPC], F32, tag="out_sb", name="out_sb")
            nc.vector.tensor_scalar_mul(out_sb[:], out2[:], 0.5)

            nc.sync.dma_start(out=d_out[:], in_=out_sb[:])

    nc.compile()
    return nc


def _get_nc():
    if "nc" not in _CACHE:
        _CACHE["nc"] = _build()
    return _CACHE["nc"]


# ---------------------------------------------------------------------------
# host-side packing + execution
# ---------------------------------------------------------------------------


def _pack_common(queries, values, Wi, bi, Wm, bm, Wq, bq, Wk, bk, Wv, bv, Wo, bo):
    f = np.float32
    queries = np.asarray(queries, f)
    values = np.asarray(values, f)

    # x_7 = concat(queries[b,q], values[b,7]) for row b*Q+q, transposed+ones row
    x7 = np.concatenate(
        [queries.reshape(BQ, DIM), np.repeat(values[:, V - 1, :], Q, axis=0)], axis=1
    )
    x7aT = np.concatenate([x7.T, np.ones((1, BQ), f)], axis=0)
    WiA = np.concatenate([np.asarray(Wi, f), (np.asarray(bi, f) + PE[0])[None, :]], 0)
    pk33 = np.concatenate([x7aT, WiA], axis=1).astype(BF16)  # (33, 192)

    WqA = np.concatenate(
        [np.asarray(Wq, f).reshape(EMB, 128), np.asarray(bq, f).reshape(1, 128)], 0
    )
    WkA = np.concatenate(
        [np.asarray(Wk, f).reshape(EMB, 128), np.asarray(bk, f).reshape(1, 128)], 0
    )
    WvA = np.concatenate(
        [np.asarray(Wv, f).reshape(EMB, 128), np.asarray(bv, f).reshape(1, 128)], 0
    )
    pk65_head = np.concatenate([WqA, WkA, WvA], axis=1).astype(BF16)  # (65, 384)

    PET1 = PE[1:].T  # (64 d, 64 l)
    WoP = np.asarray(Wo, f).transpose(1, 0, 2).reshape(EMB, 128)  # (64k, h*d)
    pk64 = np.concatenate([PET1, WoP], axis=1).astype(BF16)  # (64, 192)

    Wm = np.asarray(Wm, f)
    hmask = np.zeros((128, RH), f)
    for h in range(RH):
        hmask[h * EMB : (h + 1) * EMB, h] = 1.0
    pk128 = np.concatenate(
        [Wm[0:128, :], Wm[128:256, :], np.eye(128, dtype=f), np.ones((128, 1), f), hmask],
        axis=1,
    ).astype(BF16)  # (128, 259)

    pkb = np.stack(
        [np.asarray(bm, f).reshape(EMB), np.asarray(bo, f).reshape(EMB)], axis=1
    )  # (64, 2) f32

    return pk33, pk65_head, pk64, pk128, np.ascontiguousarray(pkb)


def kernel(
    queries,
    values,
    Wi,
    bi,
    Wm,
    bm,
    Wq,
    bq,
    Wk,
    bk,
    Wv,
    bv,
    Wo,
    bo,
    Wx,
    bl,
):
    global LAST_EXEC_TIME_NS
    from concourse.bass_utils import run_bass_kernel_spmd

    f = np.float32
    pk33, pk65_head, pk64, pk128, pkb = _pack_common(
        queries, values, Wi, bi, Wm, bm, Wq, bq, Wk, bk, Wv, bv, Wo, bo
    )
    Wx = np.asarray(Wx, f)
    bl = np.asarray(bl, f)

    # per-core slice of Wx/bl: zi, zg, zo gate blocks, CPC columns each
    gate_off = [0, 2 * UNITS, 3 * UNITS]  # zi, zg, zo starts in the 4*UNITS axis
    in_maps = []
    for c in range(N_CORES):
        cols = np.concatenate(
            [np.arange(off + c * CPC, off + (c + 1) * CPC) for off in gate_off]
        )
        WxA = np.concatenate([Wx[:, cols], bl[cols][None, :]], axis=0)
        pk65 = np.concatenate([pk65_head, WxA.astype(BF16)], axis=1)  # (65, 768)
        in_maps.append(
            {
                "pk33": np.ascontiguousarray(pk33),
                "pk65": np.ascontiguousarray(pk65),
                "pk64": np.ascontiguousarray(pk64),
                "pk128": np.ascontiguousarray(pk128),
                "pkb": pkb,
            }
        )

    nc = _get_nc()
    trace = os.environ.get("BASS_TRACE", "") not in ("", "0")
    core_ids = list(range(N_CORES))
    if trace:
        import tempfile

        tmpdir = tempfile.mkdtemp(prefix="bass_trace_")
        _CACHE["trace_dir"] = tmpdir
        try:
            res = run_bass_kernel_spmd(
                nc, in_maps, core_ids=core_ids, trace=True, tmpdir=tmpdir
            )
        except Exception as e:  # profiling infra missing: fall back untraced
            print(f"trace failed ({e!r}); rerunning without trace")
            os.environ["BASS_TRACE"] = "0"
            res = run_bass_kernel_spmd(nc, in_maps, core_ids=core_ids, trace=False)
    else:
        res = run_bass_kernel_spmd(nc, in_maps, core_ids=core_ids, trace=False)
    LAST_EXEC_TIME_NS = res.exec_time_ns

    out_full = np.concatenate([res.results[c]["out"] for c in range(N_CORES)], axis=1)
    return out_full.reshape(-1, Q, DIM).astype(f)


# revision 12
# speedup vs baseline: 1.4247x; 1.0294x over previous
"""Trainium2 Bass kernel for nn_MemoryLayerAttention_27917287424099.

Mathematical collapse of the reference RNN:
  - The conductance-ODE "pot" state receives zero external input
    (neuron_inputs = zeros), starts at the same (0, 1) pair in every one
    of the BQ*MC cells, and its update depends only on itself and
    hardcoded constants.  It therefore evolves identically in every cell
    and is a compile-time-constant scalar trajectory.
  - Only the LAST scan step's LSTM output is returned (ys[-1]), and steps
    interact only through pot, so steps 0..6's attention/LSTM outputs are
    dead code.
  - Hence the whole model == one attention + LSTM-gate step evaluated on
    x_7 = concat(queries[b,q], values[b,7]) with the memory matrix equal
    to the constant p0 (pot[...,0] after 7*2 Euler iterations) broadcast
    everywhere.
  - Of the LSTM gate pre-activation z (4*1184 cols), only zi/zg/zo's
    first 1024 columns are used (zf and the tail are dead).

Sharding: batch (128) lives on the SBUF partition dim; the replicated
attention preamble is computed on every core, and the 1024 output
columns of the LSTM matmul + gate math are sharded 128/core across the
8 cores (each core gets its own 3*128-column slice of Wx/bl).

Perf notes baked in:
  - fp32 matmuls run as LOW_HIGH double passes on trn2; all TensorE
    operands are bf16 here (single pass), PSUM accumulation stays fp32.
    Measured end-to-end error vs the f32 reference: ~5e-3.
  - each independent matmul accumulation group owns its own PSUM tile
    (two groups sharing a PSUM bank crash the device).
  - inputs arrive in 5 packed DMAs (DMA issue is serialized on SyncE at
    ~0.7us apiece, so count matters, not bytes).
  - sigmoid(x) = 0.5*(1+tanh(x/2)) keeps every ACT function in the
    exp_and_others table set: one ACT_TABLE_LOAD instead of two.
"""

import os
import numpy as np
import ml_dtypes

BF16 = ml_dtypes.bfloat16

DIM = 16
EMB = 64
ROWS = 64
RH = 2
OUT = 1024
UNITS = 1184
B, Q, V = 8, 16, 8
BQ = B * Q
DSTEPS = 2
N_CORES = 8
CPC = OUT // N_CORES  # columns per core = 128
SCALE = float(1.0 / np.sqrt(np.float32(EMB)))

# ---------------------------------------------------------------------------
# compile-time constants (derived only from constants hardcoded in the model)
# ---------------------------------------------------------------------------


def _pot_scalar():
    """p0 = pot[..., 0] as read by scan step 7 (after 14 f32 Euler steps)."""
    cond = np.array([0.07915332, 1.0334609, 1.3365093, 0.4505964], np.float32)
    mean = np.array([0.5, 0.07879465, 0.06618887, 0.0], np.float32)
    std = np.array([100.0, 100.0, 100.0, 1.0], np.float32)
    tgt = np.array([1.5931877, 1.4378392, 0.0, 0.0], np.float32)
    part = np.float32(1.5573331 / DSTEPS)

    def sig(x):
        return np.float32(1.0) / (np.float32(1.0) + np.exp(-x, dtype=np.float32))

    p = np.array([0.0, 1.0], np.float32)
    inp = np.zeros(2, np.float32)
    for _ in range((V - 1) * DSTEPS):
        pre = np.stack([inp, p, p[::-1], np.full_like(p, np.inf)], -1)
        s = sig(std * (pre - mean))
        curr = cond * s * (tgt - p[:, None])
        p = (p + curr.sum(-1, dtype=np.float32) * part).astype(np.float32)
    return float(p[0])


P0 = _pot_scalar()


def _pe_table():
    L = ROWS + 1
    pos = np.arange(L, dtype=np.float32)[:, None]
    i = np.arange(EMB)[None, :]
    ang = pos / np.power(10000.0, (2 * (i // 2)) / EMB)
    return np.where(i % 2 == 0, np.sin(ang), np.cos(ang)).astype(np.float32)


PE = _pe_table()  # (65, 64)

# packed-input column offsets
# pk33 (33, 192): x7aT | WiA
# pk65 (65, 768): WqA | WkA | WvA | WxA(384)
# pk64 (64, 192): PET1 | WoP_h0 | WoP_h1
# pk128 (128, 259): Wm_chunk0 | Wm_chunk1 | ident | ones | hmask(2)
# pkb  (64, 2) f32: bm | bo

_CACHE = {}
LAST_EXEC_TIME_NS = None


def _build():
    import concourse.bacc as bacc
    import concourse.tile as tile
    from concourse import mybir

    F32 = mybir.dt.float32
    BF = mybir.dt.bfloat16
    AF = mybir.ActivationFunctionType
    ALU = mybir.AluOpType
    AX = mybir.AxisListType

    nc = bacc.Bacc(None, target_bir_lowering=False, debug=False)

    d_pk33 = nc.declare_dram_parameter("pk33", [33, 192], BF, isOutput=False)
    d_pk65 = nc.declare_dram_parameter("pk65", [EMB + 1, 768], BF, isOutput=False)
    d_pk64 = nc.declare_dram_parameter("pk64", [EMB, 192], BF, isOutput=False)
    d_pk128 = nc.declare_dram_parameter("pk128", [128, 259], BF, isOutput=False)
    d_pkb = nc.declare_dram_parameter("pkb", [EMB, 2], F32, isOutput=False)
    d_out = nc.declare_dram_parameter("out", [BQ, CPC], F32, isOutput=True)

    with tile.TileContext(nc) as tc:
        with (
            tc.tile_pool(name="sb", bufs=1) as sb,
            tc.tile_pool(name="ps", bufs=1, space="PSUM") as ps,
        ):
            # ---- packed loads, ordered by first use --------------------
            pk33 = sb.tile([33, 192], BF, tag="pk33", name="pk33")
            nc.sync.dma_start(out=pk33[:], in_=d_pk33[:])
            pk65 = sb.tile([EMB + 1, 768], BF, tag="pk65", name="pk65")
            nc.scalar.dma_start(out=pk65[:], in_=d_pk65[:])
            pk128 = sb.tile([128, 259], BF, tag="pk128", name="pk128")
            nc.sync.dma_start(out=pk128[:], in_=d_pk128[:])
            pkb = sb.tile([EMB, 2], F32, tag="pkb", name="pkb")
            nc.sync.dma_start(out=pkb[:], in_=d_pkb[:])
            pk64 = sb.tile([EMB, 192], BF, tag="pk64", name="pk64")
            nc.gpsimd.dma_start(out=pk64[:], in_=d_pk64[:])

            x7aT = pk33[:, 0:128]
            WiA = pk33[:, 128:192]
            WqA = pk65[:, 0:128]
            WkA = pk65[:, 128:256]
            WvA = pk65[:, 256:384]
            WxA = pk65[:, 384:768]
            PET1 = pk64[:, 0:64]
            WoP = [pk64[:, 64 + h * EMB : 64 + (h + 1) * EMB] for h in range(RH)]
            WmC = [pk128[:, h * EMB : (h + 1) * EMB] for h in range(2)]
            ident = pk128[:, 128:256]
            ones = pk128[:, 256:257]
            hmask = pk128[:, 257:259]
            bm = pkb[:, 0:1]
            bo = pkb[:, 1:2]

            # warm the ACT table set early (Exp/Tanh load overlaps the DMAs)
            warm = sb.tile([128, 1], F32, tag="warm", name="warm")
            nc.vector.memset(warm[:], 0.0)
            warm2 = sb.tile([128, 1], F32, tag="warm2", name="warm2")
            nc.scalar.activation(warm2[:], warm[:], AF.Exp)

            # ---- aug0T = (x7 @ Wi + bi + PE0)^T, augmented with ones row
            emb_ps = ps.tile([EMB, BQ], F32, tag="mm", bufs=5, name="emb_ps")
            nc.tensor.matmul(emb_ps[:], lhsT=WiA, rhs=x7aT, start=True, stop=True)
            aug0T = sb.tile([EMB + 1, BQ], BF, tag="aug0T", name="aug0T")
            nc.scalar.copy(aug0T[0:EMB, :], emb_ps[:])
            nc.vector.memset(aug0T[EMB : EMB + 1, :], 1.0)

            # ---- m_vec = p0 * colsum(Wm) + bm  (per-partition, EMB rows)
            colsum_ps = ps.tile([EMB, 1], F32, tag="mm", bufs=5, name="colsum_ps")
            nc.tensor.matmul(
                colsum_ps[:], lhsT=WmC[0], rhs=ones, start=True, stop=False
            )
            nc.tensor.matmul(
                colsum_ps[:], lhsT=WmC[1], rhs=ones, start=False, stop=True
            )
            m_vec = sb.tile([EMB, 1], F32, tag="m_vec", name="m_vec")
            nc.scalar.activation(
                m_vec[:], colsum_ps[:], AF.Identity, bias=bm, scale=P0
            )

            # ---- augR = (m_vec + PE[1:].T), augmented with ones row -----
            augR = sb.tile([EMB + 1, ROWS], BF, tag="augR", name="augR")
            nc.vector.tensor_scalar_add(augR[0:EMB, :], PET1, m_vec[:])
            nc.vector.memset(augR[EMB : EMB + 1, :], 1.0)

            # ---- q / k0 / v0 -------------------------------------------
            q_ps = ps.tile([128, BQ], F32, tag="mm", bufs=5, name="q_ps")
            nc.tensor.matmul(q_ps[:], lhsT=WqA, rhs=aug0T[:], start=True, stop=True)
            qT = sb.tile([128, BQ], BF, tag="qT", name="qT")
            nc.scalar.mul(qT[:], q_ps[:], SCALE)  # fold attention scale into q

            k0_ps = ps.tile([128, BQ], F32, tag="mm", bufs=5, name="k0_ps")
            nc.tensor.matmul(k0_ps[:], lhsT=WkA, rhs=aug0T[:], start=True, stop=True)
            k0T = sb.tile([128, BQ], BF, tag="k0T", name="k0T")
            nc.vector.tensor_copy(k0T[:], k0_ps[:])

            # v0 batch-major: (128b, 128hk)
            v0_ps = ps.tile([BQ, 128], F32, tag="mm", bufs=5, name="v0_ps")
            nc.tensor.matmul(v0_ps[:], lhsT=aug0T[:], rhs=WvA, start=True, stop=True)
            v0bm = sb.tile([BQ, 128], BF, tag="v0bm", name="v0bm")
            nc.vector.tensor_copy(v0bm[:], v0_ps[:])

            # ---- K^T (k-major) and V (l-major) for the 64 memory rows ---
            kt_ps = ps.tile([128, ROWS], F32, tag="mm", bufs=5, name="kt_ps")
            nc.tensor.matmul(kt_ps[:], lhsT=WkA, rhs=augR[:], start=True, stop=True)
            ktT = sb.tile([128, ROWS], BF, tag="ktT", name="ktT")
            nc.vector.tensor_copy(ktT[:], kt_ps[:])

            vl_ps = ps.tile([ROWS, 128], F32, tag="mm", bufs=5, name="vl_ps")
            nc.tensor.matmul(vl_ps[:], lhsT=augR[:], rhs=WvA, start=True, stop=True)
            vl = sb.tile([ROWS, 128], BF, tag="vl", name="vl")
            nc.vector.tensor_copy(vl[:], vl_ps[:])

            # ---- attention logits --------------------------------------
            logR_ps = []
            for h in range(RH):
                lp = ps.tile([BQ, ROWS], F32, tag="mm", bufs=5, name=f"logR{h}")
                nc.tensor.matmul(
                    lp[:],
                    lhsT=qT[h * EMB : (h + 1) * EMB, :],
                    rhs=ktT[h * EMB : (h + 1) * EMB, :],
                    start=True,
                    stop=True,
                )
                logR_ps.append(lp)
            prod = sb.tile([128, BQ], BF, tag="prod", name="prod")
            nc.vector.tensor_mul(prod[:], qT[:], k0T[:])
            log0_ps = ps.tile([BQ, RH], F32, tag="mm", bufs=5, name="log0_ps")
            nc.tensor.matmul(log0_ps[:], lhsT=prod[:], rhs=hmask, start=True, stop=True)

            # ---- softmax over 65 positions per (b, h) -------------------
            # |logit| <= ~2 here, so no max-subtraction needed before exp
            e = sb.tile([BQ, RH, ROWS + 1], F32, tag="e", name="e")
            s0 = sb.tile([BQ, RH], F32, tag="s0", name="s0")
            sR = sb.tile([BQ, RH], F32, tag="sR", name="sR")
            for h in range(RH):
                nc.scalar.activation(
                    e[:, h, 0:1], log0_ps[:, h : h + 1], AF.Exp,
                    accum_out=s0[:, h : h + 1],
                )
                nc.scalar.activation(
                    e[:, h, 1:], logR_ps[h][:], AF.Exp, accum_out=sR[:, h : h + 1]
                )
            ssum = sb.tile([BQ, RH], F32, tag="ssum", name="ssum")
            nc.vector.tensor_add(ssum[:], s0[:], sR[:])
            rsum = sb.tile([BQ, RH], F32, tag="rsum", name="rsum")
            nc.vector.reciprocal(rsum[:], ssum[:])
            attn = sb.tile([BQ, RH, ROWS + 1], BF, tag="attn", name="attn")
            for h in range(RH):
                nc.vector.tensor_scalar_mul(
                    attn[:, h, :], e[:, h, :], rsum[:, h : h + 1]
                )

            # ---- ctx^T, laid out (64 k, 2 h, 128 b); all matmul operands
            # at base partition 0, one PSUM tile per matmul group ---------
            atT_sb = []
            for h in range(RH):
                atT_ps = ps.tile([ROWS, BQ], BF, tag="mm", bufs=5, name=f"atT{h}")
                nc.tensor.transpose(atT_ps[:], attn[:, h, 1:], ident)
                t = sb.tile([ROWS, BQ], BF, tag=f"atTs{h}", name=f"atTs{h}")
                nc.scalar.copy(t[:], atT_ps[:])
                atT_sb.append(t)
            ctxR_ps = []
            for h in range(RH):
                cp = ps.tile([EMB, BQ], F32, tag="ctx", bufs=2, name=f"ctxR{h}")
                nc.tensor.matmul(
                    cp[:],
                    lhsT=vl[:, h * EMB : (h + 1) * EMB],
                    rhs=atT_sb[h][:],
                    start=True,
                    stop=True,
                )
                ctxR_ps.append(cp)
            # l=0 term: attn0 * v0 batch-major, one full transpose
            ctx0bm = sb.tile([BQ, 128], BF, tag="ctx0bm", name="ctx0bm")
            for h in range(RH):
                nc.vector.tensor_scalar(
                    ctx0bm[:, h * EMB : (h + 1) * EMB],
                    v0bm[:, h * EMB : (h + 1) * EMB],
                    e[:, h, 0:1],
                    rsum[:, h : h + 1],
                    op0=ALU.mult,
                    op1=ALU.mult,
                )
            c0p = ps.tile([128, BQ], BF, tag="mm", bufs=5, name="ctx0T_ps")
            nc.tensor.transpose(c0p[:], ctx0bm[:], ident)
            ctx0T_sb = sb.tile([128, BQ], F32, tag="ctx0T_sb", name="ctx0T_sb")
            nc.scalar.copy(ctx0T_sb[:], c0p[:])
            ctx = sb.tile([EMB, RH, BQ], BF, tag="ctx_sb", name="ctx")
            for h in range(RH):
                nc.vector.tensor_add(
                    ctx[:, h, :],
                    ctxR_ps[h][:],
                    ctx0T_sb[h * EMB : (h + 1) * EMB, :],
                )

            # ---- o^T = sum_h Wo[h]^T ctx[h] + bo, augmented ones row ----
            oT_ps = ps.tile([EMB, BQ], F32, tag="mm", bufs=5, name="oT_ps")
            for h in range(RH):
                nc.tensor.matmul(
                    oT_ps[:],
                    lhsT=WoP[h],
                    rhs=ctx[:, h, :],
                    start=(h == 0),
                    stop=(h == RH - 1),
                )
            oTa = sb.tile([EMB + 1, BQ], BF, tag="oTa", name="oTa")
            nc.scalar.activation(oTa[0:EMB, :], oT_ps[:], AF.Identity, bias=bo)
            nc.vector.memset(oTa[EMB : EMB + 1, :], 1.0)

            # ---- z = o @ WxA + bl  (this core's 3*128 columns) ----------
            z_ps = ps.tile([BQ, 3 * CPC], F32, tag="z", bufs=1, name="z_ps")
            nc.tensor.matmul(z_ps[:], lhsT=oTa[:], rhs=WxA, start=True, stop=True)

            # ---- gates via tanh only (one ACT table set):
            # sig(x) = 0.5*(1+tanh(x/2))
            # out = sig(zo)*tanh(sig(zi)*tanh(zg))
            #     = 0.5*(t_o+1)*tanh(0.5*(t_i+1)*t_g)
            t_i = sb.tile([BQ, CPC], F32, tag="t_i", name="t_i")
            nc.scalar.activation(t_i[:], z_ps[:, 0:CPC], AF.Tanh, scale=0.5)
            t_g = sb.tile([BQ, CPC], F32, tag="t_g", name="t_g")
            nc.scalar.activation(t_g[:], z_ps[:, CPC : 2 * CPC], AF.Tanh)
            t_o = sb.tile([BQ, CPC], F32, tag="t_o", name="t_o")
            nc.scalar.activation(t_o[:], z_ps[:, 2 * CPC : 3 * CPC], AF.Tanh, scale=0.5)
            c2 = sb.tile([BQ, CPC], F32, tag="c2", name="c2")
            nc.vector.scalar_tensor_tensor(
                c2[:], t_i[:], 1.0, t_g[:], op0=ALU.add, op1=ALU.mult
            )
            tanh_c = sb.tile([BQ, CPC], F32, tag="tanh_c", name="tanh_c")
            nc.scalar.activation(tanh_c[:], c2[:], AF.Tanh, scale=0.5)
            out2 = sb.tile([BQ, CPC], F32, tag="out2", name="out2")
            nc.vector.scalar_tensor_tensor(
                out2[:], t_o[:], 1.0, tanh_c[:], op0=ALU.add, op1=ALU.mult
            )
            out_sb = sb.tile([BQ, CPC], F32, tag="out_sb", name="out_sb")
            nc.vector.tensor_scalar_mul(out_sb[:], out2[:], 0.5)

            nc.sync.dma_start(out=d_out[:], in_=out_sb[:])

    nc.compile()
    return nc


def _get_nc():
    if "nc" not in _CACHE:
        _CACHE["nc"] = _build()
    return _CACHE["nc"]


# ---------------------------------------------------------------------------
# host-side packing + execution
# ---------------------------------------------------------------------------


def _pack_common(queries, values, Wi, bi, Wm, bm, Wq, bq, Wk, bk, Wv, bv, Wo, bo):
    f = np.float32
    queries = np.asarray(queries, f)
    values = np.asarray(values, f)

    # x_7 = concat(queries[b,q], values[b,7]) for row b*Q+q, transposed+ones row
    x7 = np.concatenate(
        [queries.reshape(BQ, DIM), np.repeat(values[:, V - 1, :], Q, axis=0)], axis=1
    )
    x7aT = np.concatenate([x7.T, np.ones((1, BQ), f)], axis=0)
    WiA = np.concatenate([np.asarray(Wi, f), (np.asarray(bi, f) + PE[0])[None, :]], 0)
    pk33 = np.concatenate([x7aT, WiA], axis=1).astype(BF16)  # (33, 192)

    WqA = np.concatenate(
        [np.asarray(Wq, f).reshape(EMB, 128), np.asarray(bq, f).reshape(1, 128)], 0
    )
    WkA = np.concatenate(
        [np.asarray(Wk, f).reshape(EMB, 128), np.asarray(bk, f).reshape(1, 128)], 0
    )
    WvA = np.concatenate(
        [np.asarray(Wv, f).reshape(EMB, 128), np.asarray(bv, f).reshape(1, 128)], 0
    )
    pk65_head = np.concatenate([WqA, WkA, WvA], axis=1).astype(BF16)  # (65, 384)

    PET1 = PE[1:].T  # (64 d, 64 l)
    WoP = np.asarray(Wo, f).transpose(1, 0, 2).reshape(EMB, 128)  # (64k, h*d)
    pk64 = np.concatenate([PET1, WoP], axis=1).astype(BF16)  # (64, 192)

    Wm = np.asarray(Wm, f)
    hmask = np.zeros((128, RH), f)
    for h in range(RH):
        hmask[h * EMB : (h + 1) * EMB, h] = 1.0
    pk128 = np.concatenate(
        [Wm[0:128, :], Wm[128:256, :], np.eye(128, dtype=f), np.ones((128, 1), f), hmask],
        axis=1,
    ).astype(BF16)  # (128, 259)

    pkb = np.stack(
        [np.asarray(bm, f).reshape(EMB), np.asarray(bo, f).reshape(EMB)], axis=1
    )  # (64, 2) f32

    return pk33, pk65_head, pk64, pk128, np.ascontiguousarray(pkb)


def kernel(
    queries,
    values,
    Wi,
    bi,
    Wm,
    bm,
    Wq,
    bq,
    Wk,
    bk,
    Wv,
    bv,
    Wo,
    bo,
    Wx,
    bl,
):
    global LAST_EXEC_TIME_NS
    from concourse.bass_utils import run_bass_kernel_spmd

    f = np.float32
    pk33, pk65_head, pk64, pk128, pkb = _pack_common(
        queries, values, Wi, bi, Wm, bm, Wq, bq, Wk, bk, Wv, bv, Wo, bo
    )
    Wx = np.asarray(Wx, f)
    bl = np.asarray(bl, f)

    # per-core slice of Wx/bl: zi, zg, zo gate blocks, CPC columns each
    gate_off = [0, 2 * UNITS, 3 * UNITS]  # zi, zg, zo starts in the 4*UNITS axis
    in_maps = []
    for c in range(N_CORES):
        cols = np.concatenate(
            [np.arange(off + c * CPC, off + (c + 1) * CPC) for off in gate_off]
        )
        WxA = np.concatenate([Wx[:, cols], bl[cols][None, :]], axis=0)
        pk65 = np.concatenate([pk65_head, WxA.astype(BF16)], axis=1)  # (65, 768)
        in_maps.append(
            {
                "pk33": np.ascontiguousarray(pk33),
                "pk65": np.ascontiguousarray(pk65),
                "pk64": np.ascontiguousarray(pk64),
                "pk128": np.ascontiguousarray(pk128),
                "pkb": pkb,
            }
        )

    nc = _get_nc()
    trace = os.environ.get("BASS_TRACE", "") not in ("", "0")
    core_ids = list(range(N_CORES))
    if trace:
        import tempfile

        tmpdir = tempfile.mkdtemp(prefix="bass_trace_")
        _CACHE["trace_dir"] = tmpdir
        try:
            res = run_bass_kernel_spmd(
                nc, in_maps, core_ids=core_ids, trace=True, tmpdir=tmpdir
            )
        except Exception as e:  # profiling infra missing: fall back untraced
            print(f"trace failed ({e!r}); rerunning without trace")
            os.environ["BASS_TRACE"] = "0"
            res = run_bass_kernel_spmd(nc, in_maps, core_ids=core_ids, trace=False)
    else:
        res = run_bass_kernel_spmd(nc, in_maps, core_ids=core_ids, trace=False)
    LAST_EXEC_TIME_NS = res.exec_time_ns

    out_full = np.concatenate([res.results[c]["out"] for c in range(N_CORES)], axis=1)
    return out_full.reshape(-1, Q, DIM).astype(f)
